# revision 1
# baseline (speedup 1.0000x reference)
import sys

for _p in ("/opt/trn_rl_repo",):
    if _p not in sys.path:
        sys.path.insert(0, _p)

import numpy as np

# static model config (matches the reference)
RCUT, RS, NORM, A, Y, NI, MJ, L = 6.0, 3.0, 64.0, 4, 2, 2048, 64, 20.0
N = Y * NI            # 4096 atoms
M = Y * MJ            # 128 neighbors
MC = 20               # compacted slots per neighbor type (observed max active 18)
NCORES = 8
APC = N // NCORES     # 512 atoms per core
P = APC * 2 * MC      # 20480 pairs per core
CH = 512              # pairs per chunk (f32r full-rate needs >=256)
NCH = P // CH         # 40 chunks; first half neighbor-type 0, second type 1

_prog_cache = {}


def _t3to6(x, axis, bias=0.0):
    xa = np.moveaxis(x, axis, 0)
    s2 = np.float32(2.0 ** 0.5)
    t = np.stack([xa[0] * xa[0] - bias, s2 * xa[0] * xa[1], s2 * xa[0] * xa[2],
                  xa[1] * xa[1] - bias, s2 * xa[1] * xa[2], xa[2] * xa[2] - bias])
    return np.moveaxis(t, 0, axis).astype(np.float32)


def _build_program():
    if "nc" in _prog_cache:
        return _prog_cache["nc"], _prog_cache["names"]
    import concourse.bacc as bacc
    import concourse.mybir as mybir
    from concourse.tile import TileContext

    f32 = mybir.dt.float32
    f32r = mybir.dt.float32r
    TANH = mybir.ActivationFunctionType.Tanh

    nc = bacc.Bacc("TRN2", target_bir_lowering=False, debug=False)
    sc_d = nc.dram_tensor("sc_in", [1, P], f32, kind="ExternalInput").ap()
    wp_d = nc.dram_tensor("wp_in", [64, 448], f32, kind="ExternalInput").ap()
    bc_d = nc.dram_tensor("bc_in", [64, 6], f32, kind="ExternalInput").ap()
    e2_d = nc.dram_tensor("e2_out", [64, P], f32, kind="ExternalOutput").ap()
    h1_d = nc.dram_tensor("h1_out", [32, P], f32, kind="ExternalOutput").ap()

    with TileContext(nc) as tc:
        with (
            tc.tile_pool(name="const", bufs=1) as cpool,
            tc.tile_pool(name="h1", bufs=3) as h1pool,
            tc.tile_pool(name="t2", bufs=3) as t2pool,
            tc.tile_pool(name="t3", bufs=3) as t3pool,
            tc.tile_pool(name="e2", bufs=3) as e2pool,
            tc.tile_pool(name="p1", bufs=2, space="PSUM") as p1pool,
            tc.tile_pool(name="p2", bufs=2, space="PSUM") as p2pool,
            tc.tile_pool(name="p3", bufs=2, space="PSUM") as p3pool,
        ):
            sc_t = cpool.tile_from(sc_d)
            wp_t = cpool.tile_from(wp_d)
            bc_t = cpool.tile_from(bc_d)
            sc_r = cpool.tile([1, P], f32r)
            nc.vector.tensor_copy(sc_r[:], sc_t[:])
            wp_r = cpool.tile([64, 448], f32r)
            nc.vector.tensor_copy(wp_r[:], wp_t[:])
            for c in range(NCH):
                j = c // (NCH // 2)
                wo, bo = j * 224, j * 3
                colr = slice(c * CH, (c + 1) * CH)
                p1 = p1pool.tile([32, CH], f32)
                nc.tensor.matmul(p1[:], wp_r[0:1, wo:wo + 32],
                                 sc_r[0:1, colr])
                h1 = h1pool.tile([32, CH], f32r)
                nc.scalar.activation(h1[:], p1[:], TANH, bias=bc_t[0:32, bo:bo + 1])
                p2 = p2pool.tile([64, CH], f32)
                nc.tensor.matmul(p2[:], wp_r[0:32, wo + 32:wo + 96],
                                 h1[:])
                t2 = t2pool.tile([64, CH], f32r)
                nc.scalar.activation(t2[:], p2[:], TANH, bias=bc_t[0:64, bo + 1:bo + 2])
                p3 = p3pool.tile([64, CH], f32)
                nc.tensor.matmul(p3[:], wp_r[0:64, wo + 96:wo + 160],
                                 t2[:], start=True, stop=False)
                nc.tensor.matmul(p3[:], wp_r[0:32, wo + 160:wo + 224],
                                 h1[:], start=False, stop=True)
                t3 = t3pool.tile([64, CH], f32)
                nc.scalar.activation(t3[:], p3[:], TANH, bias=bc_t[0:64, bo + 2:bo + 3])
                e2 = e2pool.tile([64, CH], f32)
                nc.vector.tensor_add(e2[:], t3[:], t2[:].bitcast(f32))
                nc.sync.dma_start(e2_d[:, colr], e2[:])
                nc.sync.dma_start(h1_d[:, colr], h1[:].bitcast(f32))

    nc.compile()
    _prog_cache["nc"] = nc
    _prog_cache["names"] = ("sc_in", "wp_in", "bc_in", "e2_out", "h1_out")
    return nc, _prog_cache["names"]


def kernel(coord_3N, box_33, nbrs_idx, sr_mean, sr_std, eW1, eb1, eW2, eb2, eW3, eb3,
           Tbias, fW1, fb1, fW2, fb2, fWo, fbo, Ebias, **_):
    coord = np.asarray(coord_3N, np.float32)
    box = np.asarray(box_33, np.float32)
    nbrs = np.asarray(nbrs_idx)
    ibox = np.linalg.inv(box.astype(np.float64)).astype(np.float32)

    # ---- host: compaction (index prep) -------------------------------------
    d = coord[:, nbrs] - coord[:, :, None]                      # [3,N,M]
    frac = np.einsum("ab,bnm->anm", ibox, d)
    d = d - np.einsum("ab,bnm->anm", box, np.round(frac))
    r = np.sqrt((d.astype(np.float64) ** 2).sum(0) + 1e-18)
    act = (r > 1e-6) & (r < RCUT)                               # sr != 0
    cnbrs = np.empty((N, 2 * MC), np.int64)
    arange_n = np.arange(N)
    for j in range(Y):
        blk = act[:, j * MJ:(j + 1) * MJ]
        for n in range(N):
            ids = nbrs[n, j * MJ:(j + 1) * MJ][blk[n]]
            k = len(ids)
            assert k <= MC, f"active count {k} exceeds MC={MC}"
            cnbrs[n, j * MC:j * MC + k] = ids
            cnbrs[n, j * MC + k:(j + 1) * MC] = n               # self-pad -> sr=0
    # ---- host: geometry on compacted pairs ---------------------------------
    cd = coord[:, cnbrs] - coord[:, :, None]                    # [3,N,2MC]
    cfrac = np.einsum("ab,bnm->anm", ibox, cd)
    cd = (cd - np.einsum("ab,bnm->anm", box, np.round(cfrac))).astype(np.float32)
    cr = np.sqrt((cd ** 2).sum(0) + np.float32(1e-18)).astype(np.float32)
    u = (cr - RS) / (RCUT - RS)
    sw = np.where(cr < RS, np.float32(1.0),
                  np.where(cr < RCUT, ((-6.0 * u + 15.0) * u - 10.0) * u ** 3 + 1.0,
                           np.float32(0.0))).astype(np.float32)
    sr = np.where(cr > 1e-6, sw / np.maximum(cr, np.float32(1e-6)),
                  np.float32(0.0)).astype(np.float32)
    ti = arange_n // NI                                         # center type
    std_i = np.asarray(sr_std, np.float32)[ti][:, None]
    mean_i = np.asarray(sr_mean, np.float32)[ti][:, None]
    sc = ((sr - mean_i) / std_i).astype(np.float32)             # [N, 2MC]
    srn = (sr / std_i).astype(np.float32)
    xn = (cd / (cr + np.float32(1e-16))).astype(np.float32)
    R3 = np.float32(3 ** 0.5) * srn * xn
    R6 = np.float32(3.0) * srn * _t3to6(xn, 0, np.float32(1.0 / 3.0))
    RX = np.concatenate([srn[None], R3, R6], 0).astype(np.float32)  # [10,N,2MC]

    # ---- device: per-pair embedding MLP ------------------------------------
    eW1, eb1 = np.asarray(eW1, np.float32), np.asarray(eb1, np.float32)
    eW2, eb2 = np.asarray(eW2, np.float32), np.asarray(eb2, np.float32)
    eW3, eb3 = np.asarray(eW3, np.float32), np.asarray(eb3, np.float32)
    in_maps = []
    for core in range(NCORES):
        i = core // (NCORES // Y)
        a0 = core * APC
        scc = sc[a0:a0 + APC]                                   # [APC, 2MC]
        sc_flat = np.concatenate([scc[:, :MC].ravel(), scc[:, MC:].ravel()])
        wp = np.zeros((64, 448), np.float32)
        bc = np.zeros((64, 6), np.float32)
        for j in range(Y):
            o = j * 224
            wp[0, o:o + 32] = eW1[i, j, 0]
            wp[0:32, o + 32:o + 96] = eW2[i, j]
            wp[0:64, o + 96:o + 160] = eW3[i, j]
            wp[0:32, o + 160:o + 224] = eW3[i, j, 0:32] + eW3[i, j, 32:64]
            bc[0:32, j * 3] = eb1[i, j]
            bc[0:64, j * 3 + 1] = eb2[i, j]
            bc[0:64, j * 3 + 2] = eb3[i, j]
        in_maps.append({"sc_in": sc_flat.reshape(1, P).astype(np.float32),
                        "wp_in": wp, "bc_in": bc})

    nc, _ = _build_program()
    from concourse import bass_utils
    import time as _time
    _t0 = _time.perf_counter_ns()
    res = bass_utils.run_bass_kernel_spmd(nc, in_maps, core_ids=list(range(NCORES)))
    globals()["LAST_RUN_NS"] = _time.perf_counter_ns() - _t0
    results = res.results

    # ---- host: unshard embed, T/G contraction + fitting nets ---------------
    embed = np.empty((N, 2 * MC, 64), np.float32)
    for core in range(NCORES):
        e2 = results[core]["e2_out"]                            # [64, P]
        h1 = results[core]["h1_out"]                            # [32, P]
        emb = e2.T.copy()
        emb[:, 0:32] += h1.T
        emb[:, 32:64] += h1.T
        emb = emb.reshape(2, APC, MC, 64)                       # j-major
        a0 = core * APC
        embed[a0:a0 + APC, :MC] = emb[0]
        embed[a0:a0 + APC, MC:] = emb[1]

    T = np.einsum("xnm,nmw->nxw", RX, embed).astype(np.float32) / np.float32(NORM)
    T_NW = T[:, 0] + np.asarray(Tbias, np.float32)
    T3 = T[:, 1:4]
    T6 = T[:, 4:]
    G = T_NW[:, None, :] * T_NW[:, :A, None] + np.einsum("ncw,nca->naw", T3, T3[:, :, :A])
    G2 = _t3to6(T3[:, :, A:2 * A], axis=1) + T6[:, :, A:2 * A]
    G = (G + np.einsum("nca,ncw->naw", G2, T6)).astype(np.float32)
    Gf = G.reshape(Y, NI, A * 64)
    fW1, fb1 = np.asarray(fW1, np.float32), np.asarray(fb1, np.float32)
    fW2, fb2 = np.asarray(fW2, np.float32), np.asarray(fb2, np.float32)
    fWo, fbo = np.asarray(fWo, np.float32), np.asarray(fbo, np.float32)
    h = np.tanh(np.einsum("ind,idh->inh", Gf, fW1) + fb1[:, None]).astype(np.float32)
    h = (np.tanh(np.einsum("inh,ihg->ing", h, fW2) + fb2[:, None]) + h).astype(np.float32)
    out = (np.einsum("inh,iho->ino", h, fWo) + fbo[:, None]).astype(np.float32)
    energy = (out[..., 0] + np.asarray(Ebias, np.float32)[:, None]).sum(dtype=np.float32)
    return np.float32(energy)



# revision 2
# speedup vs baseline: 1.0523x; 1.0523x over previous
import sys

for _p in ("/opt/trn_rl_repo",):
    if _p not in sys.path:
        sys.path.insert(0, _p)

import numpy as np

# static model config (matches the reference)
RCUT, RS, NORM, A, Y, NI, MJ, L = 6.0, 3.0, 64.0, 4, 2, 2048, 64, 20.0
N = Y * NI            # 4096 atoms
M = Y * MJ            # 128 neighbors
MC = 20               # compacted slots per neighbor type (observed max active 18)
NCORES = 8
APC = N // NCORES     # 512 atoms per core
P = APC * 2 * MC      # 20480 pairs per core
CH = 512              # MLP pairs per chunk
NCH = P // CH         # 40 chunks; first half neighbor-type 0, second type 1
TCH = 2560            # T-stage columns per chunk (128 atoms x 20 slots)
TA = TCH // MC        # 128 atoms per T chunk
NTQ = (P // 2) // TCH # 4 T chunks per type-half

_prog_cache = {}


def _build_program():
    if "nc" in _prog_cache:
        return _prog_cache["nc"]
    import concourse.bacc as bacc
    import concourse.mybir as mybir
    from concourse.tile import TileContext

    f32 = mybir.dt.float32
    f32r = mybir.dt.float32r
    TANH = mybir.ActivationFunctionType.Tanh
    IDEN = mybir.ActivationFunctionType.Identity
    COPY = mybir.ActivationFunctionType.Copy
    MULT = mybir.AluOpType.mult
    SUB = mybir.AluOpType.subtract
    X = mybir.AxisListType.X

    nc = bacc.Bacc("TRN2", target_bir_lowering=False, debug=False)
    rx_d = nc.dram_tensor("rx_in", [10, P], f32, kind="ExternalInput").ap()
    wp_d = nc.dram_tensor("wp_in", [64, 448], f32, kind="ExternalInput").ap()
    bc_d = nc.dram_tensor("bc_in", [64, 6], f32, kind="ExternalInput").ap()
    fw1_d = nc.dram_tensor("fw1_in", [128, 512], f32, kind="ExternalInput").ap()
    fw2_d = nc.dram_tensor("fw2_in", [128, 512], f32, kind="ExternalInput").ap()
    fwo_d = nc.dram_tensor("fwo_in", [128, 2], f32, kind="ExternalInput").ap()
    fb_d = nc.dram_tensor("fb_in", [128, 4], f32, kind="ExternalInput").ap()
    cst_d = nc.dram_tensor("cst_in", [64, 4], f32, kind="ExternalInput").ap()
    out_d = nc.dram_tensor("out_out", [1, APC], f32, kind="ExternalOutput").ap()

    with TileContext(nc) as tc:
        with (
            tc.tile_pool(name="const", bufs=1) as cpool,
            tc.tile_pool(name="big", bufs=1) as bpool,
            tc.tile_pool(name="dram", bufs=1, space="DRAM") as dpool,
        ):
            wp_t = cpool.tile_from(wp_d)
            bc_t = cpool.tile_from(bc_d)
            cst_t = cpool.tile_from(cst_d)
            fw1_t = cpool.tile_from(fw1_d)
            fw2_t = cpool.tile_from(fw2_d)
            fwo_t = cpool.tile_from(fwo_d)
            fb_t = cpool.tile_from(fb_d)
            wp_r = cpool.tile([64, 448], f32r)
            nc.vector.tensor_copy(wp_r[:], wp_t[:])
            fw1_r = cpool.tile([128, 512], f32r)
            nc.vector.tensor_copy(fw1_r[:], fw1_t[:])
            fw2_r = cpool.tile([128, 512], f32r)
            nc.vector.tensor_copy(fw2_r[:], fw2_t[:])
            fwo_r = cpool.tile([128, 2], f32r)
            nc.vector.tensor_copy(fwo_r[:], fwo_t[:])

            E_all = bpool.tile([64, P], f32)
            T_all = bpool.tile([64, 10 * APC], f32)
            T0p = bpool.tile([64, APC], f32)
            G01 = bpool.tile([128, APC], f32r)
            G23 = bpool.tile([128, APC], f32r)
            Td = dpool.tile([64, 10 * APC], f32)
            G2d = dpool.tile([24, APC], f32)

            # ---- stage A: per-pair embedding MLP -> E_all [64, P] ----------
            with (
                tc.tile_pool(name="mlp", bufs=3) as mpool,
                tc.tile_pool(name="mp1", bufs=2, space="PSUM") as p1pool,
                tc.tile_pool(name="mp2", bufs=2, space="PSUM") as p2pool,
                tc.tile_pool(name="mp3", bufs=2, space="PSUM") as p3pool,
            ):
                for c in range(NCH):
                    j = c // (NCH // 2)
                    wo, bo = j * 224, j * 3
                    colr = slice(c * CH, (c + 1) * CH)
                    rx0 = mpool.tile([1, CH], f32, tag="rx0")
                    nc.sync.dma_start(rx0[:], rx_d[0:1, colr])
                    scr = mpool.tile([1, CH], f32r, tag="scr")
                    nc.vector.tensor_scalar(scr[:], rx0[:], float(NORM),
                                            cst_t[0:1, 2:3], MULT, SUB)
                    p1 = p1pool.tile([32, CH], f32)
                    nc.tensor.matmul(p1[:], wp_r[0:1, wo:wo + 32], scr[:])
                    h1 = mpool.tile([64, CH], f32r, tag="h1")
                    nc.scalar.activation(h1[0:32, :], p1[:], TANH,
                                         bias=bc_t[0:32, bo:bo + 1])
                    nc.scalar.activation(h1[32:64, :], p1[:], TANH,
                                         bias=bc_t[0:32, bo:bo + 1])
                    p2 = p2pool.tile([64, CH], f32)
                    nc.tensor.matmul(p2[:], wp_r[0:32, wo + 32:wo + 96], h1[0:32, :])
                    t2 = mpool.tile([64, CH], f32r, tag="t2")
                    nc.scalar.activation(t2[:], p2[:], TANH, bias=bc_t[0:64, bo + 1:bo + 2])
                    p3 = p3pool.tile([64, CH], f32)
                    nc.tensor.matmul(p3[:], wp_r[0:64, wo + 96:wo + 160],
                                     t2[:], start=True, stop=False)
                    nc.tensor.matmul(p3[:], wp_r[0:32, wo + 160:wo + 224],
                                     h1[0:32, :], start=False, stop=True)
                    t3 = mpool.tile([64, CH], f32, tag="t3")
                    nc.scalar.activation(t3[:], p3[:], TANH, bias=bc_t[0:64, bo + 2:bo + 3])
                    esl = E_all[:, colr]
                    nc.vector.tensor_add(esl, t3[:], t2[:].bitcast(f32))
                    nc.vector.tensor_add(esl, esl, h1[:].bitcast(f32))

            # ---- stage B: T contraction -> T_all [64, 10*APC] --------------
            # T[w, x*APC+n] = sum_p RX[x,p]*E[w,p] over the atom's 40 slots
            with tc.tile_pool(name="tst", bufs=2) as tpool:
                for x in range(10):
                    for q in range(NTQ):
                        tmp = tpool.tile([64, TA * 2 * MC], f32, tag="tmp")
                        for j in range(2):
                            cols = slice(j * (P // 2) + q * TCH,
                                         j * (P // 2) + (q + 1) * TCH)
                            rxb = tpool.tile([64, TCH], f32, tag="rxb")
                            nc.sync.dma_start(
                                rxb[:], rx_d[x:x + 1, cols].broadcast_to([64, TCH]))
                            dst = tmp[:].rearrange("w (n c) -> w n c", c=2 * MC)[
                                :, :, j * MC:(j + 1) * MC]
                            nc.vector.tensor_mul(
                                dst,
                                E_all[:, cols].rearrange("w (n s) -> w n s", s=MC),
                                rxb[:].rearrange("w (n s) -> w n s", s=MC))
                        nc.vector.reduce_sum(
                            T_all[:, x * APC + q * TA: x * APC + (q + 1) * TA],
                            tmp[:].rearrange("w (n c) -> w n c", c=2 * MC), axis=X)

            # ---- stage C: T0p, DRAM bounce, G2 build -----------------------
            nc.scalar.activation(T0p[:], T_all[:, 0:APC], IDEN, bias=cst_t[0:64, 0:1])
            nc.sync.dma_start(Td[:, 0:APC], T0p[:])
            nc.sync.dma_start(Td[:, APC:10 * APC], T_all[:, APC:10 * APC])

            i1 = [0, 0, 0, 1, 1, 2]
            i2 = [0, 1, 2, 1, 2, 2]
            with tc.tile_pool(name="g2", bufs=1) as gpool:
                V1 = gpool.tile([24, APC], f32)
                V2 = gpool.tile([24, APC], f32)
                T6s = gpool.tile([24, APC], f32)
                for a in range(4):
                    for c in range(6):
                        r = a * 6 + c
                        nc.sync.dma_start(
                            V1[r:r + 1, :],
                            Td[4 + a:5 + a, (1 + i1[c]) * APC:(2 + i1[c]) * APC])
                        nc.sync.dma_start(
                            V2[r:r + 1, :],
                            Td[4 + a:5 + a, (1 + i2[c]) * APC:(2 + i2[c]) * APC])
                        nc.sync.dma_start(
                            T6s[r:r + 1, :],
                            Td[4 + a:5 + a, (4 + c) * APC:(5 + c) * APC])
                G2a = gpool.tile([24, APC], f32)
                nc.vector.tensor_mul(G2a[:], V1[:], V2[:])
                nc.scalar.activation(G2a[:], G2a[:], COPY, scale=cst_t[0:24, 1:2])
                nc.vector.tensor_add(G2a[:], G2a[:], T6s[:])
                nc.sync.dma_start(G2d[:], G2a[:])

                # ---- stage D: G accumulation -> G01, G23 -------------------
                with tc.tile_pool(name="gacc", bufs=4) as apool:
                    for a in range(4):
                        ga = apool.tile([64, APC], f32, tag="ga")
                        for k in range(10):
                            bt = apool.tile([64, APC], f32, tag="bt")
                            if k < 4:
                                nc.sync.dma_start(
                                    bt[:],
                                    Td[a:a + 1, k * APC:(k + 1) * APC]
                                    .broadcast_to([64, APC]))
                            else:
                                nc.sync.dma_start(
                                    bt[:],
                                    G2d[a * 6 + k - 4:a * 6 + k - 3, :]
                                    .broadcast_to([64, APC]))
                            tt = T0p[:] if k == 0 else T_all[:, k * APC:(k + 1) * APC]
                            if k == 0:
                                nc.vector.tensor_mul(ga[:], tt, bt[:])
                            elif k < 9:
                                gt = apool.tile([64, APC], f32, tag="gt")
                                nc.vector.tensor_mul(gt[:], tt, bt[:])
                                nc.vector.tensor_add(ga[:], ga[:], gt[:])
                            else:
                                gt = apool.tile([64, APC], f32, tag="gt")
                                nc.vector.tensor_mul(gt[:], tt, bt[:])
                                gar = apool.tile([64, APC], f32r, tag="gar")
                                nc.vector.tensor_add(gar[:], ga[:], gt[:])
                        gdst = (G01 if a < 2 else G23)[(a % 2) * 64:(a % 2) * 64 + 64, :]
                        nc.sync.dma_start(gdst, gar[:])

            # ---- stage E: fitting net -> out [1, APC] ----------------------
            with (
                tc.tile_pool(name="fit", bufs=1) as fpool,
                tc.tile_pool(name="fp1", bufs=2, space="PSUM") as f1pool,
                tc.tile_pool(name="fp2", bufs=2, space="PSUM") as f2pool,
                tc.tile_pool(name="fpo", bufs=1, space="PSUM") as fopool,
            ):
                h1t = []
                for hb in range(2):
                    hp = f1pool.tile([128, APC], f32)
                    nc.tensor.matmul(hp[:], fw1_r[0:128, hb * 128:hb * 128 + 128],
                                     G01[:], start=True, stop=False)
                    nc.tensor.matmul(hp[:], fw1_r[0:128, 256 + hb * 128:256 + hb * 128 + 128],
                                     G23[:], start=False, stop=True)
                    ht = fpool.tile([128, APC], f32r, tag=f"h1_{hb}")
                    nc.scalar.activation(ht[:], hp[:], TANH, bias=fb_t[0:128, hb:hb + 1])
                    h1t.append(ht)
                h2t = []
                for hb in range(2):
                    hp = f2pool.tile([128, APC], f32)
                    nc.tensor.matmul(hp[:], fw2_r[0:128, hb * 128:hb * 128 + 128],
                                     h1t[0][:], start=True, stop=False)
                    nc.tensor.matmul(hp[:], fw2_r[0:128, 256 + hb * 128:256 + hb * 128 + 128],
                                     h1t[1][:], start=False, stop=True)
                    tt = fpool.tile([128, APC], f32, tag=f"t2_{hb}")
                    nc.scalar.activation(tt[:], hp[:], TANH, bias=fb_t[0:128, 2 + hb:3 + hb])
                    ht = fpool.tile([128, APC], f32r, tag=f"h2_{hb}")
                    nc.vector.tensor_add(ht[:], tt[:], h1t[hb][:].bitcast(f32))
                    h2t.append(ht)
                op = fopool.tile([1, APC], f32)
                nc.tensor.matmul(op[:], fwo_r[0:128, 0:1], h2t[0][:],
                                 start=True, stop=False)
                nc.tensor.matmul(op[:], fwo_r[0:128, 1:2], h2t[1][:],
                                 start=False, stop=True)
                ot = fpool.tile([1, APC], f32, tag="ot")
                nc.scalar.activation(ot[:], op[:], COPY)
                nc.sync.dma_start(out_d[:], ot[:])

    nc.compile()
    _prog_cache["nc"] = nc
    return nc


def kernel(coord_3N, box_33, nbrs_idx, sr_mean, sr_std, eW1, eb1, eW2, eb2, eW3, eb3,
           Tbias, fW1, fb1, fW2, fb2, fWo, fbo, Ebias, **_):
    coord = np.asarray(coord_3N, np.float32)
    box = np.asarray(box_33, np.float32)
    nbrs = np.asarray(nbrs_idx)
    ibox = np.linalg.inv(box.astype(np.float64)).astype(np.float32)

    # ---- host: geometry + compaction (index prep) --------------------------
    d = coord[:, nbrs] - coord[:, :, None]                      # [3,N,M]
    frac = np.einsum("ab,bnm->anm", ibox, d)
    d = d - np.einsum("ab,bnm->anm", box, np.round(frac))
    r = np.sqrt((d.astype(np.float64) ** 2).sum(0) + 1e-18)
    act = (r > 1e-6) & (r < RCUT)                               # sr != 0
    cnbrs = np.empty((N, 2 * MC), np.int64)
    arange_n = np.arange(N)
    for j in range(Y):
        blk = act[:, j * MJ:(j + 1) * MJ]
        assert blk.sum(1).max() <= MC, "active count exceeds MC"
        order = np.argsort(~blk, axis=1, kind="stable")[:, :MC]
        ids = np.take_along_axis(nbrs[:, j * MJ:(j + 1) * MJ], order, 1)
        m = np.take_along_axis(blk, order, 1)
        cnbrs[:, j * MC:(j + 1) * MC] = np.where(m, ids, arange_n[:, None])
    # ---- host: geometry on compacted pairs ---------------------------------
    cd = coord[:, cnbrs] - coord[:, :, None]                    # [3,N,2MC]
    cfrac = np.einsum("ab,bnm->anm", ibox, cd)
    cd = (cd - np.einsum("ab,bnm->anm", box, np.round(cfrac))).astype(np.float32)
    cr = np.sqrt((cd ** 2).sum(0) + np.float32(1e-18)).astype(np.float32)
    u = (cr - RS) / (RCUT - RS)
    sw = np.where(cr < RS, np.float32(1.0),
                  np.where(cr < RCUT, ((-6.0 * u + 15.0) * u - 10.0) * u ** 3 + 1.0,
                           np.float32(0.0))).astype(np.float32)
    sr = np.where(cr > 1e-6, sw / np.maximum(cr, np.float32(1e-6)),
                  np.float32(0.0)).astype(np.float32)
    ti = arange_n // NI                                         # center type
    sr_mean = np.asarray(sr_mean, np.float32)
    sr_std = np.asarray(sr_std, np.float32)
    std_i = sr_std[ti][:, None]
    srn = (sr / std_i).astype(np.float32)
    xn = (cd / (cr + np.float32(1e-16))).astype(np.float32)
    s2 = np.float32(2.0 ** 0.5)
    R3 = np.float32(3 ** 0.5) * srn * xn
    t6 = np.stack([xn[0] * xn[0] - np.float32(1 / 3), s2 * xn[0] * xn[1],
                   s2 * xn[0] * xn[2], xn[1] * xn[1] - np.float32(1 / 3),
                   s2 * xn[1] * xn[2], xn[2] * xn[2] - np.float32(1 / 3)])
    R6 = np.float32(3.0) * srn * t6
    RX = (np.concatenate([srn[None], R3, R6], 0) / np.float32(NORM)).astype(np.float32)

    # ---- per-core input packing --------------------------------------------
    eW1, eb1 = np.asarray(eW1, np.float32), np.asarray(eb1, np.float32)
    eW2, eb2 = np.asarray(eW2, np.float32), np.asarray(eb2, np.float32)
    eW3, eb3 = np.asarray(eW3, np.float32), np.asarray(eb3, np.float32)
    Tbias = np.asarray(Tbias, np.float32)
    fW1, fb1 = np.asarray(fW1, np.float32), np.asarray(fb1, np.float32)
    fW2, fb2 = np.asarray(fW2, np.float32), np.asarray(fb2, np.float32)
    fWo, fbo = np.asarray(fWo, np.float32), np.asarray(fbo, np.float32)
    Ebias = np.asarray(Ebias, np.float32)

    in_maps = []
    for core in range(NCORES):
        i = core // (NCORES // Y)
        a0 = core * APC
        rxc = RX[:, a0:a0 + APC, :]                             # [10, APC, 2MC]
        rx_flat = np.concatenate([rxc[:, :, :MC].reshape(10, -1),
                                  rxc[:, :, MC:].reshape(10, -1)], 1)
        wp = np.zeros((64, 448), np.float32)
        bc = np.zeros((64, 6), np.float32)
        for j in range(Y):
            o = j * 224
            wp[0, o:o + 32] = eW1[i, j, 0]
            wp[0:32, o + 32:o + 96] = eW2[i, j]
            wp[0:64, o + 96:o + 160] = eW3[i, j]
            wp[0:32, o + 160:o + 224] = eW3[i, j, 0:32] + eW3[i, j, 32:64]
            bc[0:32, j * 3] = eb1[i, j]
            bc[0:64, j * 3 + 1] = eb2[i, j]
            bc[0:64, j * 3 + 2] = eb3[i, j]
        fw1 = np.concatenate([fW1[i][0:128, :], fW1[i][128:256, :]], 1)
        fw2 = np.concatenate([fW2[i][0:128, :], fW2[i][128:256, :]], 1)
        fwo = np.stack([fWo[i][0:128, 0], fWo[i][128:256, 0]], 1)
        fb = np.stack([fb1[i][0:128], fb1[i][128:256],
                       fb2[i][0:128], fb2[i][128:256]], 1)
        cst = np.zeros((64, 4), np.float32)
        cst[:, 0] = Tbias
        cst[0:24, 1] = np.tile(np.float32([1, s2, s2, 1, s2, 1]), 4)
        cst[0, 2] = sr_mean[i] / sr_std[i]
        in_maps.append({
            "rx_in": np.ascontiguousarray(rx_flat),
            "wp_in": wp, "bc_in": bc,
            "fw1_in": np.ascontiguousarray(fw1),
            "fw2_in": np.ascontiguousarray(fw2),
            "fwo_in": np.ascontiguousarray(fwo),
            "fb_in": np.ascontiguousarray(fb),
            "cst_in": cst,
        })

    nc = _build_program()
    from concourse import bass_utils
    import time as _time
    _t0 = _time.perf_counter_ns()
    res = bass_utils.run_bass_kernel_spmd(nc, in_maps, core_ids=list(range(NCORES)))
    globals()["LAST_RUN_NS"] = _time.perf_counter_ns() - _t0
    results = res.results

    # ---- host: final energy ------------------------------------------------
    energy = np.float32(0.0)
    for core in range(NCORES):
        energy += results[core]["out_out"].sum(dtype=np.float32)
    for i in range(Y):
        energy += np.float32(NI) * (fbo[i, 0] + Ebias[i])
    return np.float32(energy)


# revision 3
# speedup vs baseline: 1.1172x; 1.0616x over previous
import sys

for _p in ("/opt/trn_rl_repo",):
    if _p not in sys.path:
        sys.path.insert(0, _p)

import numpy as np

# static model config (matches the reference)
RCUT, RS, NORM, A, Y, NI, MJ, L = 6.0, 3.0, 64.0, 4, 2, 2048, 64, 20.0
N = Y * NI            # 4096 atoms
M = Y * MJ            # 128 neighbors
MC = 20               # compacted slots per neighbor type (observed max active 18)
NCORES = 8
APC = N // NCORES     # 512 atoms per core
P = APC * 2 * MC      # 20480 pairs per core
CH = 512              # MLP pairs per chunk
NCH = P // CH         # 40 chunks; first half neighbor-type 0, second type 1
TCH = 2560            # T-stage columns per chunk (128 atoms x 20 slots)
TA = TCH // MC        # 128 atoms per T chunk
NTQ = (P // 2) // TCH # 4 T chunks per type-half

_prog_cache = {}


def _build_program():
    if "nc" in _prog_cache:
        return _prog_cache["nc"]
    import concourse.bacc as bacc
    import concourse.mybir as mybir
    from concourse.tile import TileContext

    f32 = mybir.dt.float32
    f32r = mybir.dt.float32r
    bf16 = mybir.dt.bfloat16
    TANH = mybir.ActivationFunctionType.Tanh
    IDEN = mybir.ActivationFunctionType.Identity
    COPY = mybir.ActivationFunctionType.Copy
    MULT = mybir.AluOpType.mult
    SUB = mybir.AluOpType.subtract
    X = mybir.AxisListType.X

    nc = bacc.Bacc("TRN2", target_bir_lowering=False, debug=False)
    u0_d = nc.dram_tensor("u0_in", [1, P], f32, kind="ExternalInput").ap()
    xn_d = nc.dram_tensor("xn_in", [3, P], bf16, kind="ExternalInput").ap()
    wpc_d = nc.dram_tensor("wpc_in", [64, 460], f32, kind="ExternalInput").ap()
    fwx_d = nc.dram_tensor("fwx_in", [128, 1026], bf16, kind="ExternalInput").ap()
    fb_d = nc.dram_tensor("fb_in", [128, 4], f32, kind="ExternalInput").ap()
    out_d = nc.dram_tensor("out_out", [1, APC], f32, kind="ExternalOutput").ap()

    with TileContext(nc) as tc:
        with (
            tc.tile_pool(name="const", bufs=1) as cpool,
            tc.tile_pool(name="big", bufs=1) as bpool,
            tc.tile_pool(name="dram", bufs=1, space="DRAM") as dpool,
        ):
            wpc_t = cpool.tile_from(wpc_d)
            bc_t = wpc_t[0:64, 448:454]
            cst_t = wpc_t[0:64, 454:460]
            fwx_t = cpool.tile_from(fwx_d)
            fb_t = cpool.tile_from(fb_d)
            wp_r = cpool.tile([64, 448], f32r)
            nc.vector.tensor_copy(wp_r[:], wpc_t[0:64, 0:448])
            fw1_r = fwx_t[0:128, 0:512]
            fw2_r = fwx_t[0:128, 512:1024]
            fwo_r = fwx_t[0:128, 1024:1026]

            E_all = bpool.tile([64, P], f32)
            T_all = bpool.tile([64, 10 * APC], f32)
            T0p = bpool.tile([64, APC], f32)
            G01 = bpool.tile([128, APC], bf16)
            G23 = bpool.tile([128, APC], bf16)
            Td = dpool.tile([64, 10 * APC], f32)
            G2d = dpool.tile([24, APC], f32)
            rxs = dpool.tile([10, P], f32)

            # ---- stage G0: device-side R3/R6 rows -> rxs[1:10] -------------
            S3 = float(3.0 ** 0.5)
            GCH = 2560
            with tc.tile_pool(name="geo", bufs=2) as geopool:
                for g in range(P // GCH):
                    cols = slice(g * GCH, (g + 1) * GCH)
                    sb6 = geopool.tile([6, GCH], f32, tag="sb6")
                    nc.sync.dma_start(sb6[:], u0_d[0:1, cols].broadcast_to([6, GCH]))
                    v1 = geopool.tile([6, GCH], bf16, tag="v1")
                    nc.sync.dma_start(v1[0:3, :], xn_d[0:1, cols].broadcast_to([3, GCH]))
                    nc.sync.dma_start(v1[3:5, :], xn_d[1:2, cols].broadcast_to([2, GCH]))
                    nc.sync.dma_start(v1[5:6, :], xn_d[2:3, cols])
                    v2 = geopool.tile([6, GCH], bf16, tag="v2")
                    nc.sync.dma_start(v2[0:3, :], xn_d[0:3, cols])
                    nc.sync.dma_start(v2[3:4, :], xn_d[1:2, cols])
                    nc.sync.dma_start(v2[4:6, :], xn_d[2:3, cols].broadcast_to([2, GCH]))
                    t6 = geopool.tile([6, GCH], f32, tag="t6")
                    nc.vector.tensor_mul(t6[:], v1[:], v2[:])
                    r3 = geopool.tile([3, GCH], f32, tag="r3")
                    nc.vector.scalar_tensor_tensor(r3[:], v2[0:3, :], S3,
                                                   sb6[0:3, :], MULT, MULT)
                    nc.sync.dma_start(rxs[1:4, cols], r3[:])
                    nc.scalar.activation(t6[:], t6[:], IDEN,
                                         scale=wpc_t[0:6, 457:458], bias=wpc_t[0:6, 458:459])
                    nc.vector.scalar_tensor_tensor(t6[:], t6[:], 3.0,
                                                   sb6[:], MULT, MULT)
                    nc.sync.dma_start(rxs[4:10, cols], t6[:])

            # ---- stage A: per-pair embedding MLP -> E_all [64, P] ----------
            with (
                tc.tile_pool(name="mlp", bufs=3) as mpool,
                tc.tile_pool(name="mp1", bufs=2, space="PSUM") as p1pool,
                tc.tile_pool(name="mp2", bufs=2, space="PSUM") as p2pool,
                tc.tile_pool(name="mp3", bufs=2, space="PSUM") as p3pool,
            ):
                for c in range(NCH):
                    j = c // (NCH // 2)
                    wo, bo = j * 224, j * 3
                    colr = slice(c * CH, (c + 1) * CH)
                    rx0 = mpool.tile([1, CH], f32, tag="rx0")
                    nc.sync.dma_start(rx0[:], u0_d[0:1, colr])
                    scr = mpool.tile([1, CH], f32r, tag="scr")
                    nc.vector.tensor_scalar(scr[:], rx0[:], float(NORM),
                                            wpc_t[0:1, 456:457], MULT, SUB)
                    p1 = p1pool.tile([32, CH], f32)
                    nc.tensor.matmul(p1[:], wp_r[0:1, wo:wo + 32], scr[:])
                    h1 = mpool.tile([64, CH], f32r, tag="h1")
                    nc.scalar.activation(h1[0:32, :], p1[:], TANH,
                                         bias=wpc_t[0:32, 448 + bo:449 + bo])
                    nc.scalar.activation(h1[32:64, :], p1[:], TANH,
                                         bias=wpc_t[0:32, 448 + bo:449 + bo])
                    p2 = p2pool.tile([64, CH], f32)
                    nc.tensor.matmul(p2[:], wp_r[0:32, wo + 32:wo + 96], h1[0:32, :])
                    t2 = mpool.tile([64, CH], f32r, tag="t2")
                    nc.scalar.activation(t2[:], p2[:], TANH, bias=wpc_t[0:64, 449 + bo:450 + bo])
                    p3 = p3pool.tile([64, CH], f32)
                    nc.tensor.matmul(p3[:], wp_r[0:64, wo + 96:wo + 160],
                                     t2[:], start=True, stop=False)
                    nc.tensor.matmul(p3[:], wp_r[0:32, wo + 160:wo + 224],
                                     h1[0:32, :], start=False, stop=True)
                    t3 = mpool.tile([64, CH], f32, tag="t3")
                    nc.scalar.activation(t3[:], p3[:], TANH, bias=wpc_t[0:64, 450 + bo:451 + bo])
                    esl = E_all[:, colr]
                    nc.vector.tensor_add(esl, t3[:], t2[:].bitcast(f32))
                    nc.vector.tensor_add(esl, esl, h1[:].bitcast(f32))

            # ---- stage B: T contraction -> T_all [64, 10*APC] --------------
            # T[w, x*APC+n] = sum_p RX[x,p]*E[w,p] over the atom's 40 slots
            with tc.tile_pool(name="tst", bufs=2) as tpool:
                for x in range(10):
                    for q in range(NTQ):
                        tmp = tpool.tile([64, TA * 2 * MC], f32, tag="tmp")
                        for j in range(2):
                            cols = slice(j * (P // 2) + q * TCH,
                                         j * (P // 2) + (q + 1) * TCH)
                            rxb = tpool.tile([64, TCH], f32, tag="rxb")
                            rsrc = u0_d[0:1, cols] if x == 0 else rxs[x:x + 1, cols]
                            nc.sync.dma_start(rxb[:], rsrc.broadcast_to([64, TCH]))
                            dst = tmp[:].rearrange("w (n c) -> w n c", c=2 * MC)[
                                :, :, j * MC:(j + 1) * MC]
                            nc.vector.tensor_mul(
                                dst,
                                E_all[:, cols].rearrange("w (n s) -> w n s", s=MC),
                                rxb[:].rearrange("w (n s) -> w n s", s=MC))
                        nc.vector.reduce_sum(
                            T_all[:, x * APC + q * TA: x * APC + (q + 1) * TA],
                            tmp[:].rearrange("w (n c) -> w n c", c=2 * MC), axis=X)

            # ---- stage C: T0p, DRAM bounce, G2 build -----------------------
            nc.scalar.activation(T0p[:], T_all[:, 0:APC], IDEN, bias=wpc_t[0:64, 454:455])
            nc.sync.dma_start(Td[:, 0:APC], T0p[:])
            nc.sync.dma_start(Td[:, APC:10 * APC], T_all[:, APC:10 * APC])

            i1 = [0, 0, 0, 1, 1, 2]
            i2 = [0, 1, 2, 1, 2, 2]
            with tc.tile_pool(name="g2", bufs=1) as gpool:
                V1 = gpool.tile([24, APC], f32)
                V2 = gpool.tile([24, APC], f32)
                T6s = gpool.tile([24, APC], f32)
                for a in range(4):
                    for c in range(6):
                        r = a * 6 + c
                        nc.sync.dma_start(
                            V1[r:r + 1, :],
                            Td[4 + a:5 + a, (1 + i1[c]) * APC:(2 + i1[c]) * APC])
                        nc.sync.dma_start(
                            V2[r:r + 1, :],
                            Td[4 + a:5 + a, (1 + i2[c]) * APC:(2 + i2[c]) * APC])
                        nc.sync.dma_start(
                            T6s[r:r + 1, :],
                            Td[4 + a:5 + a, (4 + c) * APC:(5 + c) * APC])
                G2a = gpool.tile([24, APC], f32)
                nc.vector.tensor_mul(G2a[:], V1[:], V2[:])
                nc.scalar.activation(G2a[:], G2a[:], COPY, scale=wpc_t[0:24, 455:456])
                nc.vector.tensor_add(G2a[:], G2a[:], T6s[:])
                nc.sync.dma_start(G2d[:], G2a[:])

                # ---- stage D: G accumulation -> G01, G23 -------------------
                with tc.tile_pool(name="gacc", bufs=4) as apool:
                    for a in range(4):
                        ga = apool.tile([64, APC], f32, tag="ga")
                        for k in range(10):
                            bt = apool.tile([64, APC], f32, tag="bt")
                            if k < 4:
                                nc.sync.dma_start(
                                    bt[:],
                                    Td[a:a + 1, k * APC:(k + 1) * APC]
                                    .broadcast_to([64, APC]))
                            else:
                                nc.sync.dma_start(
                                    bt[:],
                                    G2d[a * 6 + k - 4:a * 6 + k - 3, :]
                                    .broadcast_to([64, APC]))
                            tt = T0p[:] if k == 0 else T_all[:, k * APC:(k + 1) * APC]
                            if k == 0:
                                nc.vector.tensor_mul(ga[:], tt, bt[:])
                            elif k < 9:
                                gt = apool.tile([64, APC], f32, tag="gt")
                                nc.vector.tensor_mul(gt[:], tt, bt[:])
                                nc.vector.tensor_add(ga[:], ga[:], gt[:])
                            else:
                                gt = apool.tile([64, APC], f32, tag="gt")
                                nc.vector.tensor_mul(gt[:], tt, bt[:])
                                gar = apool.tile([64, APC], bf16, tag="gar")
                                nc.vector.tensor_add(gar[:], ga[:], gt[:])
                        gdst = (G01 if a < 2 else G23)[(a % 2) * 64:(a % 2) * 64 + 64, :]
                        nc.sync.dma_start(gdst, gar[:])

            # ---- stage E: fitting net -> out [1, APC] ----------------------
            with (
                tc.tile_pool(name="fit", bufs=1) as fpool,
                tc.tile_pool(name="fp1", bufs=2, space="PSUM") as f1pool,
                tc.tile_pool(name="fp2", bufs=2, space="PSUM") as f2pool,
                tc.tile_pool(name="fpo", bufs=1, space="PSUM") as fopool,
            ):
                h1t = []
                for hb in range(2):
                    hp = f1pool.tile([128, APC], f32)
                    nc.tensor.matmul(hp[:], fw1_r[0:128, hb * 128:hb * 128 + 128],
                                     G01[:], start=True, stop=False)
                    nc.tensor.matmul(hp[:], fw1_r[0:128, 256 + hb * 128:256 + hb * 128 + 128],
                                     G23[:], start=False, stop=True)
                    ht = fpool.tile([128, APC], bf16, tag=f"h1_{hb}")
                    nc.scalar.activation(ht[:], hp[:], TANH, bias=fb_t[0:128, hb:hb + 1])
                    h1t.append(ht)
                h2t = []
                for hb in range(2):
                    hp = f2pool.tile([128, APC], f32)
                    nc.tensor.matmul(hp[:], fw2_r[0:128, hb * 128:hb * 128 + 128],
                                     h1t[0][:], start=True, stop=False)
                    nc.tensor.matmul(hp[:], fw2_r[0:128, 256 + hb * 128:256 + hb * 128 + 128],
                                     h1t[1][:], start=False, stop=True)
                    tt = fpool.tile([128, APC], f32, tag=f"t2_{hb}")
                    nc.scalar.activation(tt[:], hp[:], TANH, bias=fb_t[0:128, 2 + hb:3 + hb])
                    ht = fpool.tile([128, APC], bf16, tag=f"h2_{hb}")
                    nc.vector.tensor_add(ht[:], tt[:], h1t[hb][:])
                    h2t.append(ht)
                op = fopool.tile([1, APC], f32)
                nc.tensor.matmul(op[:], fwo_r[0:128, 0:1], h2t[0][:],
                                 start=True, stop=False)
                nc.tensor.matmul(op[:], fwo_r[0:128, 1:2], h2t[1][:],
                                 start=False, stop=True)
                ot = fpool.tile([1, APC], f32, tag="ot")
                nc.scalar.activation(ot[:], op[:], COPY)
                nc.sync.dma_start(out_d[:], ot[:])

    nc.compile()
    _prog_cache["nc"] = nc
    return nc


def kernel(coord_3N, box_33, nbrs_idx, sr_mean, sr_std, eW1, eb1, eW2, eb2, eW3, eb3,
           Tbias, fW1, fb1, fW2, fb2, fWo, fbo, Ebias, **_):
    coord = np.asarray(coord_3N, np.float32)
    box = np.asarray(box_33, np.float32)
    nbrs = np.asarray(nbrs_idx)
    ibox = np.linalg.inv(box.astype(np.float64)).astype(np.float32)

    # ---- host: geometry + compaction (index prep) --------------------------
    diag_box = np.allclose(box, np.diag(np.diag(box)))
    bd = np.diag(box)[:, None, None]
    d = coord[:, nbrs] - coord[:, :, None]                      # [3,N,M]
    if diag_box:
        d = d - bd * np.round(d / bd)
    else:
        frac = np.einsum("ab,bnm->anm", ibox, d)
        d = d - np.einsum("ab,bnm->anm", box, np.round(frac))
    r2 = (d ** 2).sum(0)
    act = (r2 > np.float32(1e-12)) & (r2 < np.float32(RCUT * RCUT))
    cnbrs = np.empty((N, 2 * MC), np.int64)
    arange_n = np.arange(N)
    for j in range(Y):
        blk = act[:, j * MJ:(j + 1) * MJ]
        assert blk.sum(1).max() <= MC, "active count exceeds MC"
        order = np.argsort(~blk, axis=1, kind="stable")[:, :MC]
        ids = np.take_along_axis(nbrs[:, j * MJ:(j + 1) * MJ], order, 1)
        m = np.take_along_axis(blk, order, 1)
        cnbrs[:, j * MC:(j + 1) * MC] = np.where(m, ids, arange_n[:, None])
    # ---- host: geometry on compacted pairs ---------------------------------
    cd = coord[:, cnbrs] - coord[:, :, None]                    # [3,N,2MC]
    if diag_box:
        cd = (cd - bd * np.round(cd / bd)).astype(np.float32)
    else:
        cfrac = np.einsum("ab,bnm->anm", ibox, cd)
        cd = (cd - np.einsum("ab,bnm->anm", box, np.round(cfrac))).astype(np.float32)
    cr = np.sqrt((cd ** 2).sum(0) + np.float32(1e-18)).astype(np.float32)
    u = (cr - RS) / (RCUT - RS)
    sw = np.where(cr < RS, np.float32(1.0),
                  np.where(cr < RCUT, ((-6.0 * u + 15.0) * u - 10.0) * u ** 3 + 1.0,
                           np.float32(0.0))).astype(np.float32)
    sr = np.where(cr > 1e-6, sw / np.maximum(cr, np.float32(1e-6)),
                  np.float32(0.0)).astype(np.float32)
    ti = arange_n // NI                                         # center type
    sr_mean = np.asarray(sr_mean, np.float32)
    sr_std = np.asarray(sr_std, np.float32)
    std_i = sr_std[ti][:, None]
    srn = (sr / std_i).astype(np.float32)
    xn = (cd / (cr + np.float32(1e-16))).astype(np.float32)
    s2 = np.float32(2.0 ** 0.5)
    u0 = (srn / np.float32(NORM)).astype(np.float32)
    RX = np.concatenate([u0[None], xn], 0).astype(np.float32)   # [4, N, 2MC]

    # ---- per-core input packing --------------------------------------------
    eW1, eb1 = np.asarray(eW1, np.float32), np.asarray(eb1, np.float32)
    eW2, eb2 = np.asarray(eW2, np.float32), np.asarray(eb2, np.float32)
    eW3, eb3 = np.asarray(eW3, np.float32), np.asarray(eb3, np.float32)
    Tbias = np.asarray(Tbias, np.float32)
    fW1, fb1 = np.asarray(fW1, np.float32), np.asarray(fb1, np.float32)
    fW2, fb2 = np.asarray(fW2, np.float32), np.asarray(fb2, np.float32)
    fWo, fbo = np.asarray(fWo, np.float32), np.asarray(fbo, np.float32)
    Ebias = np.asarray(Ebias, np.float32)

    in_maps = []
    for core in range(NCORES):
        i = core // (NCORES // Y)
        a0 = core * APC
        import ml_dtypes
        rxc = RX[:, a0:a0 + APC, :]                             # [4, APC, 2MC]
        rx_flat = np.concatenate([rxc[:, :, :MC].reshape(4, -1),
                                  rxc[:, :, MC:].reshape(4, -1)], 1)
        u0_flat = np.ascontiguousarray(rx_flat[0:1])
        xn_flat = rx_flat[1:4].astype(ml_dtypes.bfloat16)
        wp = np.zeros((64, 448), np.float32)
        bc = np.zeros((64, 6), np.float32)
        for j in range(Y):
            o = j * 224
            wp[0, o:o + 32] = eW1[i, j, 0]
            wp[0:32, o + 32:o + 96] = eW2[i, j]
            wp[0:64, o + 96:o + 160] = eW3[i, j]
            wp[0:32, o + 160:o + 224] = eW3[i, j, 0:32] + eW3[i, j, 32:64]
            bc[0:32, j * 3] = eb1[i, j]
            bc[0:64, j * 3 + 1] = eb2[i, j]
            bc[0:64, j * 3 + 2] = eb3[i, j]
        fwx = np.concatenate(
            [np.concatenate([fW1[i][0:128, :], fW1[i][128:256, :]], 1),
             np.concatenate([fW2[i][0:128, :], fW2[i][128:256, :]], 1),
             np.stack([fWo[i][0:128, 0], fWo[i][128:256, 0]], 1)],
            1).astype(ml_dtypes.bfloat16)
        fb = np.stack([fb1[i][0:128], fb1[i][128:256],
                       fb2[i][0:128], fb2[i][128:256]], 1)
        cst = np.zeros((64, 6), np.float32)
        cst[:, 0] = Tbias
        cst[0:24, 1] = np.tile(np.float32([1, s2, s2, 1, s2, 1]), 4)
        cst[0, 2] = sr_mean[i] / sr_std[i]
        cst[0:6, 3] = np.float32([1, s2, s2, 1, s2, 1])
        cst[0:6, 4] = np.float32([-1 / 3, 0, 0, -1 / 3, 0, -1 / 3])
        wpc = np.zeros((64, 460), np.float32)
        wpc[:, 0:448] = wp
        wpc[:, 448:454] = bc
        wpc[:, 454:460] = cst
        in_maps.append({
            "u0_in": u0_flat,
            "xn_in": np.ascontiguousarray(xn_flat),
            "wpc_in": wpc,
            "fwx_in": np.ascontiguousarray(fwx),
            "fb_in": np.ascontiguousarray(fb),
        })

    nc = _build_program()
    from concourse import bass_utils
    import time as _time
    _t0 = _time.perf_counter_ns()
    res = bass_utils.run_bass_kernel_spmd(nc, in_maps, core_ids=list(range(NCORES)))
    globals()["LAST_RUN_NS"] = _time.perf_counter_ns() - _t0
    results = res.results

    # ---- host: final energy ------------------------------------------------
    energy = np.float32(0.0)
    for core in range(NCORES):
        energy += results[core]["out_out"].sum(dtype=np.float32)
    for i in range(Y):
        energy += np.float32(NI) * (fbo[i, 0] + Ebias[i])
    return np.float32(energy)
